# revision 11
# baseline (speedup 1.0000x reference)
"""Gemma2 decoder layer on 8 trn2 NeuronCores.

Strategy: data-parallel over tokens. The 4096 tokens (B=2 x S=2048) are
split into 8 contiguous shards of 512 (4 shards per batch). Each core runs
the full layer for its 512 tokens with full (unsharded) weights streamed
from HBM. Attention needs keys/values for up to 1024 earlier tokens, so
each core redundantly computes K/V for a 1536-token window (own shard +
1024-token halo, zero-padded at batch start). No collectives.

All matmuls are bf16 x bf16 -> fp32 PSUM. Activations are kept
feature-major ([feature, token]) so weight matrices (stored [in, out] in
HBM) are directly the stationary lhsT operand. RMSNorm scale vectors
(1 + w) are folded into adjacent weight matrices on the host; per-token
inv-rms factors are applied on-chip via full-partition stats matmuls
(ones/weighted lhsT -> every partition holds the sum). Softmax needs no
max-subtraction: logits are softcapped to [-50, 50], and the additive
mask is shifted by -50 so exp arguments lie in [-100, 0].
"""

import os
import sys
import types

sys.path.insert(0, "/opt/trn_rl_repo")

import numpy as np
import ml_dtypes


# ---------------------------------------------------------------------------
# NTFF profile hook shim (the image's antenv lacks axon_hooks).
# ---------------------------------------------------------------------------
def _install_ntff_shim():
    if "antenv.axon_hooks" in sys.modules:
        return
    try:
        import antenv
        mod = types.ModuleType("antenv.axon_hooks")
        holder = [None]
        mod.set_axon_ntff_profile_hook = lambda h: holder.__setitem__(0, h)
        mod.get_axon_ntff_profile_hook = lambda: holder[0]
        sys.modules["antenv.axon_hooks"] = mod
        antenv.axon_hooks = mod
        from trn_agent_boot.trn_boot import _ntff_profile_via_ctypes
        mod.set_axon_ntff_profile_hook(
            _ntff_profile_via_ctypes("/opt/axon/libaxon_pjrt.so"))
    except Exception:
        pass


_install_ntff_shim()

import concourse.bass as bass
import concourse.tile as tile
from concourse import bacc, mybir
from concourse.bass_utils import run_bass_kernel_spmd
from concourse.masks import make_identity

# ---------------------------------------------------------------------------
# Model constants (Gemma2-9B layer; hardcoded per problem spec)
# ---------------------------------------------------------------------------
HID = 3584
H = 16
KV = 8
DH = 256
INTER = 14336
WINDOW = 1024
LOGIT_CAP = 50.0
SCALING = 256.0 ** -0.5
EPS = 1e-6
ROPE_BASE = 10000.0
B, S = 2, 2048

N_CORES = 8
T_OWN = 512          # tokens per core
T_KVP = 1536         # kv window (own + 1024 halo, left-padded)
KC = HID // 128      # 28 hid chunks
IC = INTER // 128    # 112 inter chunks
KT = T_KVP // 128    # 12 kv-token chunks
HC = H * DH // 128   # 32 chunks of q/attn dims
NEG = -10000.0       # additive penalty for invalid positions

BF16 = mybir.dt.bfloat16
F32 = mybir.dt.float32
nbf16 = ml_dtypes.bfloat16
ts = bass.ts

AF = mybir.ActivationFunctionType
ALU = mybir.AluOpType

LAST_EXEC_NS = None
TRACE = os.environ.get("KERNEL_TRACE", "1") == "1"


# ---------------------------------------------------------------------------
# Device program
# ---------------------------------------------------------------------------
def _build_program():
    nc = bacc.Bacc("TRN2", target_bir_lowering=False, debug=False,
                   enable_asserts=False, num_devices=N_CORES)

    def din(name, shape, dt=BF16):
        return nc.dram_tensor(name, shape, dt, kind="ExternalInput").ap()

    def dout(name, shape, dt=F32):
        return nc.dram_tensor(name, shape, dt, kind="ExternalOutput").ap()

    # inputs (per-core)
    xkv = din("xkv", [128, KC, T_KVP])            # x, feature-major, bf16
    xown = din("xown", [128, KC, T_OWN], F32)     # own x, feature-major, fp32
    wq = din("wq", [HC, 128, KC, 128])            # lhsT-tile-major layouts
    wk = din("wk", [2 * KV, 128, KC, 128])
    wv = din("wv", [2 * KV, 128, KC, 128])
    wo = din("wo", [KC, 128, HC, 128])
    wg = din("wg", [IC, 128, KC, 128])
    wu = din("wu", [IC, 128, KC, 128])
    wd = din("wd", [KC, 128, IC, 128])
    iw_pa = din("iw_pa", [128, KC, 128])          # 1/(1+w_pa)^2, repl. cols
    iw_pf = din("iw_pf", [128, KC, 128])
    cos_q = din("cos_q", [128, T_OWN])
    sin_q = din("sin_q", [128, T_OWN])
    cos_k = din("cos_k", [128, T_KVP])            # bf16, for attention K
    sin_k = din("sin_k", [128, T_KVP])
    cos_ko = din("cos_ko", [128, T_OWN], F32)     # fp32, for K output rope
    sin_ko = din("sin_ko", [128, T_OWN], F32)
    mask = din("mask", [128, KT, T_OWN], F32)     # [k, q] additive, -50 shift

    # outputs (per-core, fp32, feature-major; host transposes)
    k_out = dout("k_out", [128, 2 * KV, T_OWN])
    v_out = dout("v_out", [128, 2 * KV, T_OWN])
    r_out = dout("r_out", [128, KC, T_OWN])
    x_out = dout("x_out", [128, KC, T_OWN])

    with tile.TileContext(nc) as tc:
        _run = tc.tile_pool  # shorthand

        with _run(name="const", bufs=1) as const, \
             _run(name="dscr", bufs=1, space="DRAM") as dscr:
            # DRAM scratch for q/k/v between phases (pool-tracked for deps)
            kscr = dscr.tile([KV, 128, 2, T_KVP], BF16, tag="kscr")
            vscr = dscr.tile([KV, 128, 2, T_KVP], BF16, tag="vscr")
            qscr = dscr.tile([KV, 128, 4, T_OWN], BF16, tag="qscr")
            ones = const.tile([128, 128], BF16)
            nc.any.memset(ones[:], 1.0)
            ident = const.tile([128, 128], BF16)
            make_identity(nc, ident)
            epsb = const.tile([128, 1], F32)
            nc.any.memset(epsb[:], EPS)

            # =========== phases 1+2: in_ln, then K/V/Q projections =========
            with _run(name="p12big", bufs=1) as pbig:
                xk = pbig.tile([128, KC, T_KVP], BF16)
                nc.sync.dma_start(xk[:], xkv[:])
                inv1 = pbig.tile([128, T_KVP], BF16, tag="inv1")

                with _run(name="p1t", bufs=3) as p1t, \
                     _run(name="p1ps", bufs=1, space="PSUM") as p1ps:
                    ssq = p1ps.tile([128, T_KVP], F32)
                    for c in range(KC):
                        sq = p1t.tile([128, T_KVP], BF16, tag="sq")
                        nc.scalar.activation(sq[:], xk[:, c], AF.Square)
                        for n in range(T_KVP // 512):
                            nc.tensor.matmul(ssq[:, ts(n, 512)], ones[:],
                                             sq[:, ts(n, 512)],
                                             start=(c == 0), stop=(c == KC - 1))
                    rms = p1t.tile([128, T_KVP], F32, tag="rms")
                    nc.scalar.activation(rms[:], ssq[:], AF.Sqrt,
                                         scale=1.0 / HID, bias=epsb[:])
                    rcp = p1t.tile([128, T_KVP], F32, tag="rcp")
                    nc.vector.reciprocal(rcp[:], rms[:])
                    nc.vector.tensor_copy(inv1[:], rcp[:])  # -> bf16

                # x_norm in place: xk *= inv1 (broadcast over KC)
                nc.vector.tensor_tensor(
                    xk[:], xk[:],
                    inv1[:, None, :].to_broadcast([128, KC, T_KVP]), ALU.mult)

                csk = pbig.tile([128, T_KVP], BF16, tag="csk")
                snk = pbig.tile([128, T_KVP], BF16, tag="snk")
                cso = pbig.tile([128, T_OWN], F32, tag="cso")
                sno = pbig.tile([128, T_OWN], F32, tag="sno")
                csq = pbig.tile([128, T_OWN], BF16, tag="csq")
                snq = pbig.tile([128, T_OWN], BF16, tag="snq")
                nc.sync.dma_start(csk[:], cos_k[:])
                nc.sync.dma_start(snk[:], sin_k[:])
                nc.sync.dma_start(cso[:], cos_ko[:])
                nc.sync.dma_start(sno[:], sin_ko[:])
                nc.sync.dma_start(csq[:], cos_q[:])
                nc.sync.dma_start(snq[:], sin_q[:])

                with _run(name="p2w", bufs=3) as p2w, \
                     _run(name="p2t", bufs=1) as p2t, \
                     _run(name="p2ps", bufs=2, space="PSUM") as p2ps:

                    def mm_chunks(ps_t, wslab, rhs, T):
                        for c in range(KC):
                            for n in range(T // 512):
                                nc.tensor.matmul(
                                    ps_t[:, ts(n, 512)], wslab[:, c],
                                    rhs[:, c, ts(n, 512)],
                                    start=(c == 0), stop=(c == KC - 1))

                    def rope(dst, a, b2, cs, sn, tmp, sign_first=-1.0):
                        # dst0 = a*cs - b2*sn ; dst1 = b2*cs + a*sn
                        d0, d1 = dst
                        nc.vector.tensor_tensor(d0, a, cs, ALU.mult)
                        nc.vector.tensor_tensor(tmp, b2, sn, ALU.mult)
                        nc.vector.tensor_sub(d0, d0, tmp)
                        nc.vector.tensor_tensor(d1, b2, cs, ALU.mult)
                        nc.vector.tensor_tensor(tmp, a, sn, ALU.mult)
                        nc.vector.tensor_add(d1, d1, tmp)

                    for h in range(KV):
                        # ---- K head: matmul -> bf16(+fp32 own) -> rope ----
                        kfb = p2t.tile([128, 2, T_KVP], BF16, tag="kfb")
                        kfo = p2t.tile([128, 2, T_OWN], F32, tag="kfo")
                        for m in range(2):
                            wsl = p2w.tile([128, KC, 128], BF16, tag="wsl")
                            nc.sync.dma_start(wsl[:], wk[2 * h + m])
                            pp = p2ps.tile([128, T_KVP], F32, tag="pkv")
                            mm_chunks(pp, wsl, xk, T_KVP)
                            nc.vector.tensor_copy(kfb[:, m], pp[:])
                            nc.scalar.copy(kfo[:, m], pp[:, T_KVP - T_OWN:])
                        # attention copy (bf16 rope over full window)
                        kb = p2t.tile([128, 2, T_KVP], BF16, tag="kb")
                        tmpb = p2t.tile([128, T_KVP], BF16, tag="tmpb")
                        rope((kb[:, 0], kb[:, 1]), kfb[:, 0], kfb[:, 1],
                             csk[:], snk[:], tmpb[:])
                        nc.sync.dma_start(kscr[h], kb[:])
                        # output copy (fp32 rope over own tokens)
                        kro = p2t.tile([128, 2, T_OWN], F32, tag="kro")
                        tmpo = p2t.tile([128, T_OWN], F32, tag="tmpo")
                        rope((kro[:, 0], kro[:, 1]), kfo[:, 0], kfo[:, 1],
                             cso[:], sno[:], tmpo[:])
                        nc.sync.dma_start(k_out[:, 2 * h:2 * h + 2, :], kro[:])

                        # ---- V head ---------------------------------------
                        vb = p2t.tile([128, 2, T_KVP], BF16, tag="vb")
                        vfo = p2t.tile([128, T_OWN], F32, tag="vfo")
                        for m in range(2):
                            wsl = p2w.tile([128, KC, 128], BF16, tag="wsl")
                            nc.sync.dma_start(wsl[:], wv[2 * h + m])
                            pp = p2ps.tile([128, T_KVP], F32, tag="pkv")
                            mm_chunks(pp, wsl, xk, T_KVP)
                            nc.vector.tensor_copy(vb[:, m], pp[:])
                            nc.scalar.copy(vfo[:], pp[:, T_KVP - T_OWN:])
                            nc.sync.dma_start(v_out[:, 2 * h + m, :], vfo[:])
                        nc.sync.dma_start(vscr[h], vb[:])

                        # ---- Q pair (heads 2h, 2h+1) ----------------------
                        qfb = p2t.tile([128, 4, T_OWN], BF16, tag="qfb")
                        for m in range(4):
                            wsl = p2w.tile([128, KC, 128], BF16, tag="wslq")
                            nc.sync.dma_start(wsl[:], wq[4 * h + m])
                            pq = p2ps.tile([128, T_OWN], F32, tag="pq")
                            mm_chunks(pq, wsl, xk[:, :, T_KVP - T_OWN:], T_OWN)
                            nc.vector.tensor_copy(qfb[:, m], pq[:])
                        qb = p2t.tile([128, 4, T_OWN], BF16, tag="qb")
                        tq = p2t.tile([128, T_OWN], BF16, tag="tq")
                        for qh in range(2):
                            rope((qb[:, 2 * qh], qb[:, 2 * qh + 1]),
                                 qfb[:, 2 * qh], qfb[:, 2 * qh + 1],
                                 csq[:], snq[:], tq[:])
                        nc.sync.dma_start(qscr[h], qb[:])

            # =========== phase 3: attention -> DRAM ====================
            ascr = dscr.tile([128, HC, T_OWN], BF16, tag="ascr")
            with _run(name="p3acc", bufs=1) as p3acc, \
                 _run(name="p3c", bufs=1) as p3c, \
                 _run(name="p3h", bufs=2) as p3h, \
                 _run(name="p3t", bufs=3) as p3t, \
                 _run(name="p3ps", bufs=2, space="PSUM") as p3ps, \
                 _run(name="p3pa", bufs=1, space="PSUM") as p3pa:
                attn = p3acc.tile([128, HC, T_OWN], BF16)
                msk = p3c.tile([128, KT, T_OWN], F32)
                nc.sync.dma_start(msk[:], mask[:])

                for h in range(KV):
                    kb = p3h.tile([128, 2, T_KVP], BF16, tag="kb")
                    nc.sync.dma_start(kb[:], kscr[h])
                    vb = p3h.tile([128, 2, T_KVP], BF16, tag="vb")
                    nc.sync.dma_start(vb[:], vscr[h])
                    qb = p3h.tile([128, 4, T_OWN], BF16, tag="qb")
                    nc.sync.dma_start(qb[:], qscr[h])

                    # v -> token-major via PE transpose
                    vt = p3h.tile([128, KT, 2, 128], BF16, tag="vt")
                    for t in range(KT):
                        pt = p3ps.tile([128, 2, 128], BF16, tag="pt")
                        for d in range(2):
                            nc.tensor.transpose(
                                pt[:, d], vb[:, d, ts(t, 128)], ident[:])
                        nc.vector.tensor_copy(vt[:, t], pt[:])

                    for qh in range(2):
                        pa = p3pa.tile([128, 2, T_OWN], F32, tag="pa")
                        pss = p3pa.tile([128, T_OWN], F32, tag="pss")
                        for t in range(KT):
                            ps_s = p3ps.tile([128, T_OWN], F32, tag="ps")
                            for d in range(2):
                                nc.tensor.matmul(
                                    ps_s[:], kb[:, d, ts(t, 128)],
                                    qb[:, 2 * qh + d],
                                    start=(d == 0), stop=(d == 1))
                            sc = p3t.tile([128, T_OWN], F32, tag="sc")
                            nc.scalar.activation(sc[:], ps_s[:], AF.Tanh,
                                                 scale=1.0 / LOGIT_CAP)
                            nc.vector.scalar_tensor_tensor(
                                sc[:], sc[:], LOGIT_CAP, msk[:, t],
                                ALU.mult, ALU.add)
                            ex = p3t.tile([128, T_OWN], BF16, tag="ex")
                            nc.scalar.activation(ex[:], sc[:], AF.Exp)
                            for d in range(2):
                                nc.tensor.matmul(
                                    pa[:, d], vt[:, t, d], ex[:],
                                    start=(t == 0), stop=(t == KT - 1))
                            nc.tensor.matmul(pss[:], ones[:], ex[:],
                                             start=(t == 0),
                                             stop=(t == KT - 1))
                        inv = p3t.tile([128, T_OWN], F32, tag="inv")
                        nc.vector.reciprocal(inv[:], pss[:])
                        hh = 2 * h + qh
                        for d in range(2):
                            nc.vector.tensor_tensor(
                                attn[:, 2 * hh + d], pa[:, d], inv[:],
                                ALU.mult)
                nc.sync.dma_start(ascr[:], attn[:])

            # ===== phases 4-6 =============================================
            NQ = 4
            ICQ = IC // NQ
            at_scr = dscr.tile([KC, 128, T_OWN], F32, tag="at_scr")
            with _run(name="p46", bufs=1) as p46:
                r2n = p46.tile([128, KC, T_OWN], BF16, tag="r2n")
                macc = p46.tile([128, KC, T_OWN], F32, tag="macc")
                inv_a = p46.tile([128, T_OWN], F32, tag="inv_a")

                # --- 4a: o_proj -> at (DRAM) + post-attn stats ------------
                with _run(name="p4a", bufs=1) as p4a, \
                     _run(name="p4w", bufs=3) as p4w, \
                     _run(name="p4t", bufs=3) as p4t, \
                     _run(name="p4ps", bufs=3, space="PSUM") as p4ps, \
                     _run(name="p4pss", bufs=1, space="PSUM") as p4pss:
                    iwpa = p4a.tile([128, KC, 128], BF16, tag="iwpa")
                    nc.sync.dma_start(iwpa[:], iw_pa[:])
                    attn = p4a.tile([128, HC, T_OWN], BF16, tag="attn")
                    nc.sync.dma_start(attn[:], ascr[:])
                    pssq = p4pss.tile([128, T_OWN], F32)
                    for m in range(KC):
                        wsl = p4w.tile([128, HC, 128], BF16, tag="wo")
                        nc.sync.dma_start(wsl[:], wo[m])
                        pp = p4ps.tile([128, T_OWN], F32, tag="po")
                        for c in range(HC):
                            nc.tensor.matmul(pp[:], wsl[:, c], attn[:, c],
                                             start=(c == 0),
                                             stop=(c == HC - 1))
                        atc = p4t.tile([128, T_OWN], F32, tag="atc")
                        nc.scalar.copy(atc[:], pp[:])
                        nc.sync.dma_start(at_scr[m], atc[:])
                        sq = p4t.tile([128, T_OWN], BF16, tag="sq4")
                        nc.scalar.activation(sq[:], pp[:], AF.Square)
                        nc.tensor.matmul(pssq[:], iwpa[:, m], sq[:],
                                         start=(m == 0), stop=(m == KC - 1))
                    rms = p4t.tile([128, T_OWN], F32, tag="rms4")
                    nc.scalar.activation(rms[:], pssq[:], AF.Sqrt,
                                         scale=1.0 / HID, bias=epsb[:])
                    nc.vector.reciprocal(inv_a[:], rms[:])

                # --- 4b: residual, r_out, pre-ff norm, r2n ----------------
                with _run(name="p4b", bufs=1) as p4b, \
                     _run(name="p4bt", bufs=3) as p4bt, \
                     _run(name="p4bps", bufs=1, space="PSUM") as p4bps:
                    xo = p4b.tile([128, KC, T_OWN], F32, tag="xo")
                    nc.sync.dma_start(xo[:], xown[:])
                    ps2 = p4bps.tile([128, T_OWN], F32)
                    for m in range(KC):
                        atc = p4bt.tile([128, T_OWN], F32, tag="atc2")
                        nc.sync.dma_start(atc[:], at_scr[m])
                        nc.vector.tensor_tensor(atc[:], atc[:], inv_a[:],
                                                ALU.mult)
                        nc.vector.tensor_add(xo[:, m], xo[:, m], atc[:])
                        sq = p4bt.tile([128, T_OWN], BF16, tag="sq5")
                        nc.scalar.activation(sq[:], xo[:, m], AF.Square)
                        nc.tensor.matmul(ps2[:], ones[:], sq[:],
                                         start=(m == 0), stop=(m == KC - 1))
                    nc.sync.dma_start(r_out[:], xo[:])
                    rms2 = p4bt.tile([128, T_OWN], F32, tag="rms5")
                    nc.scalar.activation(rms2[:], ps2[:], AF.Sqrt,
                                         scale=1.0 / HID, bias=epsb[:])
                    inv2 = p4bt.tile([128, T_OWN], F32, tag="inv5")
                    nc.vector.reciprocal(inv2[:], rms2[:])
                    nc.vector.tensor_tensor(
                        r2n[:], xo[:],
                        inv2[:, None, :].to_broadcast([128, KC, T_OWN]),
                        ALU.mult)

                # --- 5+6: MLP in 4 inter-quarters -------------------------
                with _run(name="p5q", bufs=1) as p5q, \
                     _run(name="p5w", bufs=3) as p5w, \
                     _run(name="p6w", bufs=2) as p6w, \
                     _run(name="p5t", bufs=3) as p5t, \
                     _run(name="p5ps", bufs=2, space="PSUM") as p5ps, \
                     _run(name="p6ps", bufs=3, space="PSUM") as p6ps:
                    for q in range(NQ):
                        itr = p5q.tile([128, ICQ, T_OWN], BF16, tag="itr")
                        for mi in range(ICQ):
                            m = q * ICQ + mi
                            wg_s = p5w.tile([128, KC, 128], BF16, tag="wg")
                            nc.sync.dma_start(wg_s[:], wg[m])
                            wu_s = p5w.tile([128, KC, 128], BF16, tag="wu")
                            nc.sync.dma_start(wu_s[:], wu[m])
                            pg = p5ps.tile([128, T_OWN], F32, tag="pg")
                            pu = p5ps.tile([128, T_OWN], F32, tag="pu")
                            for c in range(KC):
                                nc.tensor.matmul(pg[:], wg_s[:, c], r2n[:, c],
                                                 start=(c == 0),
                                                 stop=(c == KC - 1))
                            for c in range(KC):
                                nc.tensor.matmul(pu[:], wu_s[:, c], r2n[:, c],
                                                 start=(c == 0),
                                                 stop=(c == KC - 1))
                            gl = p5t.tile([128, T_OWN], BF16, tag="gl")
                            nc.scalar.activation(gl[:], pg[:],
                                                 AF.Gelu_apprx_tanh)
                            ub = p5t.tile([128, T_OWN], BF16, tag="ub")
                            nc.vector.tensor_copy(ub[:], pu[:])
                            nc.vector.tensor_tensor(itr[:, mi], gl[:], ub[:],
                                                    ALU.mult)
                        for m in range(KC):
                            wsl = p6w.tile([128, ICQ, 128], BF16, tag="wd")
                            nc.sync.dma_start(wsl[:], wd[m, :, ts(q, ICQ), :])
                            pp = p6ps.tile([128, T_OWN], F32, tag="pd")
                            for c in range(ICQ):
                                nc.tensor.matmul(pp[:], wsl[:, c], itr[:, c],
                                                 start=(c == 0),
                                                 stop=(c == ICQ - 1))
                            if q == 0:
                                nc.scalar.copy(macc[:, m], pp[:])
                            else:
                                nc.vector.tensor_add(macc[:, m], macc[:, m],
                                                     pp[:])

                    # post-ff norm over macc
                    with _run(name="p6t", bufs=3) as p6t, \
                         _run(name="p6big", bufs=1) as p6big, \
                         _run(name="p6pss", bufs=1, space="PSUM") as p6pss:
                        iwpf = p6big.tile([128, KC, 128], BF16, tag="iwpf")
                        nc.sync.dma_start(iwpf[:], iw_pf[:])
                        ps3 = p6pss.tile([128, T_OWN], F32)
                        for m in range(KC):
                            sq = p6t.tile([128, T_OWN], BF16, tag="sq6")
                            nc.scalar.activation(sq[:], macc[:, m], AF.Square)
                            nc.tensor.matmul(ps3[:], iwpf[:, m], sq[:],
                                             start=(m == 0), stop=(m == KC - 1))
                        rms3 = p6t.tile([128, T_OWN], F32, tag="rms6")
                        nc.scalar.activation(rms3[:], ps3[:], AF.Sqrt,
                                             scale=1.0 / HID, bias=epsb[:])
                        inv3 = p6t.tile([128, T_OWN], F32, tag="inv6")
                        nc.vector.reciprocal(inv3[:], rms3[:])
                        nc.vector.tensor_tensor(
                            macc[:], macc[:],
                            inv3[:, None, :].to_broadcast([128, KC, T_OWN]),
                            ALU.mult)
                        nc.sync.dma_start(x_out[:], macc[:])

    nc.compile()
    return nc


_PROGRAM = None


def _get_program():
    global _PROGRAM
    if _PROGRAM is None:
        _PROGRAM = _build_program()
    return _PROGRAM


# ---------------------------------------------------------------------------
# Host-side input prep
# ---------------------------------------------------------------------------
def _fm(x):
    """[T, D] -> feature-major [128, D//128, T]"""
    D = x.shape[1]
    return np.ascontiguousarray(
        x.T.reshape(D // 128, 128, -1).transpose(1, 0, 2))


def _wtiles(w):
    """[K, M] -> [M//128, 128, K//128, 128] lhsT-tile-major"""
    K, M = w.shape
    return np.ascontiguousarray(
        w.reshape(K // 128, 128, M // 128, 128).transpose(2, 1, 0, 3))


def kernel(hidden_states, positions, in_ln_w, post_attn_ln_w, pre_ff_ln_w,
           post_ff_ln_w, q_w, k_w, v_w, o_w, gate_w, up_w, down_w):
    global LAST_EXEC_NS
    nc = _get_program()

    f32 = np.float32
    hs = np.asarray(hidden_states, f32)
    pos = np.asarray(positions, np.int32)

    # ---- shared (weight) inputs, computed once ----
    s_in = (1.0 + np.asarray(in_ln_w, f32))[:, None]
    wq_h = _wtiles((np.asarray(q_w, f32) * s_in * SCALING).astype(nbf16))
    wk_h = _wtiles((np.asarray(k_w, f32) * s_in).astype(nbf16))
    wv_h = _wtiles((np.asarray(v_w, f32) * s_in).astype(nbf16))
    wo_h = _wtiles((np.asarray(o_w, f32) *
                    (1.0 + np.asarray(post_attn_ln_w, f32))[None, :]
                    ).astype(nbf16))
    s_ff = (1.0 + np.asarray(pre_ff_ln_w, f32))[:, None]
    wg_h = _wtiles((np.asarray(gate_w, f32) * s_ff).astype(nbf16))
    wu_h = _wtiles((np.asarray(up_w, f32) * s_ff).astype(nbf16))
    wd_h = _wtiles((np.asarray(down_w, f32) *
                    (1.0 + np.asarray(post_ff_ln_w, f32))[None, :]
                    ).astype(nbf16))

    def iw2(w):
        v = (1.0 / (1.0 + np.asarray(w, f32)) ** 2).reshape(KC, 128)
        return np.ascontiguousarray(
            np.broadcast_to(v.T[:, :, None], (128, KC, 128))).astype(nbf16)

    iwpa_h = iw2(post_attn_ln_w)
    iwpf_h = iw2(post_ff_ln_w)

    inv_freq = 1.0 / (ROPE_BASE ** (np.arange(0, DH, 2, dtype=f32) / DH))

    in_maps = []
    metas = []
    for c in range(N_CORES):
        b, s = divmod(c, N_CORES // B)
        o0, o1 = T_OWN * s, T_OWN * (s + 1)
        kv0 = o1 - T_KVP  # may be negative -> zero pad
        xo = hs[b, o0:o1]                               # [512, HID]
        xv = np.zeros((T_KVP, HID), f32)
        src0 = max(kv0, 0)
        xv[src0 - kv0:] = hs[b, src0:o1]
        pq = pos[b, o0:o1].astype(f32)                  # [512]
        pk = np.zeros(T_KVP, f32)
        pk[src0 - kv0:] = pos[b, src0:o1].astype(f32)

        ang_q = pq[:, None] * inv_freq[None, :]         # [512, 128]
        ang_k = pk[:, None] * inv_freq[None, :]

        # mask[k, q]: valid iff kpos>=0 & kpos<=qpos & qpos-kpos<WINDOW
        kpos = np.arange(kv0, o1)
        qpos = np.arange(o0, o1)
        d = qpos[None, :] - kpos[:, None]
        valid = (kpos[:, None] >= 0) & (d >= 0) & (d < WINDOW)
        m = np.where(valid, -LOGIT_CAP, NEG - LOGIT_CAP).astype(f32)
        m3 = np.ascontiguousarray(
            m.reshape(KT, 128, T_OWN).transpose(1, 0, 2))

        in_maps.append({
            "xkv": _fm(xv).astype(nbf16),
            "xown": _fm(xo),
            "wq": wq_h, "wk": wk_h, "wv": wv_h, "wo": wo_h,
            "wg": wg_h, "wu": wu_h, "wd": wd_h,
            "iw_pa": iwpa_h, "iw_pf": iwpf_h,
            "cos_q": np.ascontiguousarray(np.cos(ang_q).T).astype(nbf16),
            "sin_q": np.ascontiguousarray(np.sin(ang_q).T).astype(nbf16),
            "cos_k": np.ascontiguousarray(np.cos(ang_k).T).astype(nbf16),
            "sin_k": np.ascontiguousarray(np.sin(ang_k).T).astype(nbf16),
            "cos_ko": np.ascontiguousarray(np.cos(ang_q).T).astype(f32),
            "sin_ko": np.ascontiguousarray(np.sin(ang_q).T).astype(f32),
            "mask": m3,
        })
        metas.append((b, o0, o1))

    res = run_bass_kernel_spmd(nc, in_maps, core_ids=list(range(N_CORES)),
                               trace=TRACE,
                               trace_cores=[0] if TRACE else None)
    LAST_EXEC_NS = res.exec_time_ns

    # ---- assemble full outputs ----
    x_full = np.zeros((B, S, HID), f32)
    r_full = np.zeros((B, S, HID), f32)
    kv_full = np.zeros((2, B, S, KV, DH), f32)

    def unfm(t):
        # [128, C, T] -> [T, 128*C]
        return t.transpose(1, 0, 2).reshape(-1, t.shape[2]).T

    for c in range(N_CORES):
        b, o0, o1 = metas[c]
        r = res.results[c]
        x_full[b, o0:o1] = unfm(r["x_out"])
        r_full[b, o0:o1] = unfm(r["r_out"])
        kv_full[0, b, o0:o1] = unfm(r["k_out"]).reshape(T_OWN, KV, DH)
        kv_full[1, b, o0:o1] = unfm(r["v_out"]).reshape(T_OWN, KV, DH)

    return x_full, r_full, kv_full


# revision 13
# speedup vs baseline: 1.0146x; 1.0146x over previous
"""Gemma2 decoder layer on 8 trn2 NeuronCores.

Strategy: data-parallel over tokens. The 4096 tokens (B=2 x S=2048) are
split into 8 contiguous shards of 512 (4 shards per batch). Each core runs
the full layer for its 512 tokens with full (unsharded) weights streamed
from HBM. Attention needs keys/values for up to 1024 earlier tokens, so
each core redundantly computes K/V for a 1536-token window (own shard +
1024-token halo, zero-padded at batch start). No collectives.

All matmuls are bf16 x bf16 -> fp32 PSUM. Activations are kept
feature-major ([feature, token]) so weight matrices (stored [in, out] in
HBM) are directly the stationary lhsT operand. RMSNorm scale vectors
(1 + w) are folded into adjacent weight matrices on the host; per-token
inv-rms factors are applied on-chip via full-partition stats matmuls
(ones/weighted lhsT -> every partition holds the sum). Softmax needs no
max-subtraction: logits are softcapped to [-50, 50], and the additive
mask is shifted by -50 so exp arguments lie in [-100, 0].
"""

import os
import sys
import types

sys.path.insert(0, "/opt/trn_rl_repo")

import numpy as np
import ml_dtypes


# ---------------------------------------------------------------------------
# NTFF profile hook shim (the image's antenv lacks axon_hooks).
# ---------------------------------------------------------------------------
def _install_ntff_shim():
    if "antenv.axon_hooks" in sys.modules:
        return
    try:
        import antenv
        mod = types.ModuleType("antenv.axon_hooks")
        holder = [None]
        mod.set_axon_ntff_profile_hook = lambda h: holder.__setitem__(0, h)
        mod.get_axon_ntff_profile_hook = lambda: holder[0]
        sys.modules["antenv.axon_hooks"] = mod
        antenv.axon_hooks = mod
        from trn_agent_boot.trn_boot import _ntff_profile_via_ctypes
        mod.set_axon_ntff_profile_hook(
            _ntff_profile_via_ctypes("/opt/axon/libaxon_pjrt.so"))
    except Exception:
        pass


_install_ntff_shim()

import concourse.bass as bass
import concourse.tile as tile
from concourse import bacc, mybir
from concourse.bass_utils import run_bass_kernel_spmd
from concourse.masks import make_identity

# ---------------------------------------------------------------------------
# Model constants (Gemma2-9B layer; hardcoded per problem spec)
# ---------------------------------------------------------------------------
HID = 3584
H = 16
KV = 8
DH = 256
INTER = 14336
WINDOW = 1024
LOGIT_CAP = 50.0
SCALING = 256.0 ** -0.5
EPS = 1e-6
ROPE_BASE = 10000.0
B, S = 2, 2048

N_CORES = 8
T_OWN = 512          # tokens per core
T_KVP = 1536         # kv window (own + 1024 halo, left-padded)
KC = HID // 128      # 28 hid chunks
IC = INTER // 128    # 112 inter chunks
KT = T_KVP // 128    # 12 kv-token chunks
HC = H * DH // 128   # 32 chunks of q/attn dims
NEG = -10000.0       # additive penalty for invalid positions

BF16 = mybir.dt.bfloat16
F32 = mybir.dt.float32
nbf16 = ml_dtypes.bfloat16
ts = bass.ts

AF = mybir.ActivationFunctionType
ALU = mybir.AluOpType

LAST_EXEC_NS = None
TRACE = os.environ.get("KERNEL_TRACE", "1") == "1"


# ---------------------------------------------------------------------------
# Device program
# ---------------------------------------------------------------------------
def _build_program():
    nc = bacc.Bacc("TRN2", target_bir_lowering=False, debug=False,
                   enable_asserts=False, num_devices=N_CORES)

    def din(name, shape, dt=BF16):
        return nc.dram_tensor(name, shape, dt, kind="ExternalInput").ap()

    def dout(name, shape, dt=F32):
        return nc.dram_tensor(name, shape, dt, kind="ExternalOutput").ap()

    # inputs (per-core)
    xkv = din("xkv", [128, KC, T_KVP])            # x, feature-major, bf16
    xown = din("xown", [128, KC, T_OWN], F32)     # own x, feature-major, fp32
    wq = din("wq", [HC, 128, KC, 128])            # lhsT-tile-major layouts
    wk = din("wk", [2 * KV, 128, KC, 128])
    wv = din("wv", [2 * KV, 128, KC, 128])
    wo = din("wo", [KC, 128, HC, 128])
    wg = din("wg", [IC, 128, KC, 128])
    wu = din("wu", [IC, 128, KC, 128])
    wd = din("wd", [KC, 128, IC, 128])
    iw_pa = din("iw_pa", [128, KC, 128])          # 1/(1+w_pa)^2, repl. cols
    iw_pf = din("iw_pf", [128, KC, 128])
    cos_q = din("cos_q", [128, T_OWN])
    sin_q = din("sin_q", [128, T_OWN])
    cos_k = din("cos_k", [128, T_KVP])            # bf16, for attention K
    sin_k = din("sin_k", [128, T_KVP])
    cos_ko = din("cos_ko", [128, T_OWN], F32)     # fp32, for K output rope
    sin_ko = din("sin_ko", [128, T_OWN], F32)
    mask = din("mask", [128, KT, T_OWN], F32)     # [k, q] additive, -50 shift

    # outputs (per-core, fp32, feature-major; host transposes)
    k_out = dout("k_out", [128, 2 * KV, T_OWN])
    v_out = dout("v_out", [128, 2 * KV, T_OWN])
    r_out = dout("r_out", [128, KC, T_OWN])
    x_out = dout("x_out", [128, KC, T_OWN])

    with tile.TileContext(nc) as tc:
        _run = tc.tile_pool  # shorthand

        with _run(name="const", bufs=1) as const, \
             _run(name="dscr", bufs=1, space="DRAM") as dscr:
            # DRAM scratch for q/k/v between phases (pool-tracked for deps)
            kscr = dscr.tile([KV, 128, 2, T_KVP], BF16, tag="kscr")
            vscr = dscr.tile([KV, 128, 2, T_KVP], BF16, tag="vscr")
            qscr = dscr.tile([KV, 128, 4, T_OWN], BF16, tag="qscr")
            ones = const.tile([128, 128], BF16)
            nc.any.memset(ones[:], 1.0)
            ident = const.tile([128, 128], BF16)
            make_identity(nc, ident)
            epsb = const.tile([128, 1], F32)
            nc.any.memset(epsb[:], EPS)

            # =========== phases 1+2: in_ln, then K/V/Q projections =========
            with _run(name="p12big", bufs=1) as pbig:
                xk = pbig.tile([128, KC, T_KVP], BF16)
                for c in range(KC):
                    nc.sync.dma_start(xk[:, c], xkv[:, c])
                inv1 = pbig.tile([128, T_KVP], BF16, tag="inv1")

                with _run(name="p1t", bufs=3) as p1t, \
                     _run(name="p1ps", bufs=1, space="PSUM") as p1ps:
                    ssq = p1ps.tile([128, T_KVP], F32)
                    for c in range(KC):
                        sq = p1t.tile([128, T_KVP], BF16, tag="sq")
                        nc.scalar.activation(sq[:], xk[:, c], AF.Square)
                        for n in range(T_KVP // 512):
                            nc.tensor.matmul(ssq[:, ts(n, 512)], ones[:],
                                             sq[:, ts(n, 512)],
                                             start=(c == 0), stop=(c == KC - 1))
                    rms = p1t.tile([128, T_KVP], F32, tag="rms")
                    nc.scalar.activation(rms[:], ssq[:], AF.Sqrt,
                                         scale=1.0 / HID, bias=epsb[:])
                    rcp = p1t.tile([128, T_KVP], F32, tag="rcp")
                    nc.vector.reciprocal(rcp[:], rms[:])
                    nc.vector.tensor_copy(inv1[:], rcp[:])  # -> bf16

                # x_norm in place: xk *= inv1 (per chunk)
                for c in range(KC):
                    nc.vector.tensor_tensor(xk[:, c], xk[:, c], inv1[:],
                                            ALU.mult)

                csk = pbig.tile([128, T_KVP], BF16, tag="csk")
                snk = pbig.tile([128, T_KVP], BF16, tag="snk")
                cso = pbig.tile([128, T_OWN], F32, tag="cso")
                sno = pbig.tile([128, T_OWN], F32, tag="sno")
                csq = pbig.tile([128, T_OWN], BF16, tag="csq")
                snq = pbig.tile([128, T_OWN], BF16, tag="snq")
                nc.sync.dma_start(csk[:], cos_k[:])
                nc.sync.dma_start(snk[:], sin_k[:])
                nc.sync.dma_start(cso[:], cos_ko[:])
                nc.sync.dma_start(sno[:], sin_ko[:])
                nc.sync.dma_start(csq[:], cos_q[:])
                nc.sync.dma_start(snq[:], sin_q[:])

                with _run(name="p2w", bufs=3) as p2w, \
                     _run(name="p2t", bufs=1) as p2t, \
                     _run(name="p2ps", bufs=2, space="PSUM") as p2ps:

                    def mm_chunks(ps_t, wslab, rhs, T):
                        for c in range(KC):
                            for n in range(T // 512):
                                nc.tensor.matmul(
                                    ps_t[:, ts(n, 512)], wslab[:, c],
                                    rhs[:, c, ts(n, 512)],
                                    start=(c == 0), stop=(c == KC - 1))

                    def rope(dst, a, b2, cs, sn, tmp, sign_first=-1.0):
                        # dst0 = a*cs - b2*sn ; dst1 = b2*cs + a*sn
                        d0, d1 = dst
                        nc.vector.tensor_tensor(d0, a, cs, ALU.mult)
                        nc.vector.tensor_tensor(tmp, b2, sn, ALU.mult)
                        nc.vector.tensor_sub(d0, d0, tmp)
                        nc.vector.tensor_tensor(d1, b2, cs, ALU.mult)
                        nc.vector.tensor_tensor(tmp, a, sn, ALU.mult)
                        nc.vector.tensor_add(d1, d1, tmp)

                    for h in range(KV):
                        # ---- K head: matmul -> bf16(+fp32 own) -> rope ----
                        kfb = p2t.tile([128, 2, T_KVP], BF16, tag="kfb")
                        kfo = p2t.tile([128, 2, T_OWN], F32, tag="kfo")
                        for m in range(2):
                            wsl = p2w.tile([128, KC, 128], BF16, tag="wsl")
                            nc.sync.dma_start(wsl[:], wk[2 * h + m])
                            pp = p2ps.tile([128, T_KVP], F32, tag="pkv")
                            mm_chunks(pp, wsl, xk, T_KVP)
                            nc.vector.tensor_copy(kfb[:, m], pp[:])
                            nc.scalar.copy(kfo[:, m], pp[:, T_KVP - T_OWN:])
                        # attention copy (bf16 rope over full window)
                        kb = p2t.tile([128, 2, T_KVP], BF16, tag="kb")
                        tmpb = p2t.tile([128, T_KVP], BF16, tag="tmpb")
                        rope((kb[:, 0], kb[:, 1]), kfb[:, 0], kfb[:, 1],
                             csk[:], snk[:], tmpb[:])
                        nc.sync.dma_start(kscr[h], kb[:])
                        # output copy (fp32 rope over own tokens)
                        kro = p2t.tile([128, 2, T_OWN], F32, tag="kro")
                        tmpo = p2t.tile([128, T_OWN], F32, tag="tmpo")
                        rope((kro[:, 0], kro[:, 1]), kfo[:, 0], kfo[:, 1],
                             cso[:], sno[:], tmpo[:])
                        nc.sync.dma_start(k_out[:, 2 * h:2 * h + 2, :], kro[:])

                        # ---- V head ---------------------------------------
                        vb = p2t.tile([128, 2, T_KVP], BF16, tag="vb")
                        vfo = p2t.tile([128, T_OWN], F32, tag="vfo")
                        for m in range(2):
                            wsl = p2w.tile([128, KC, 128], BF16, tag="wsl")
                            nc.sync.dma_start(wsl[:], wv[2 * h + m])
                            pp = p2ps.tile([128, T_KVP], F32, tag="pkv")
                            mm_chunks(pp, wsl, xk, T_KVP)
                            nc.vector.tensor_copy(vb[:, m], pp[:])
                            nc.scalar.copy(vfo[:], pp[:, T_KVP - T_OWN:])
                            nc.sync.dma_start(v_out[:, 2 * h + m, :], vfo[:])
                        nc.sync.dma_start(vscr[h], vb[:])

                        # ---- Q pair (heads 2h, 2h+1) ----------------------
                        qfb = p2t.tile([128, 4, T_OWN], BF16, tag="qfb")
                        for m in range(4):
                            wsl = p2w.tile([128, KC, 128], BF16, tag="wslq")
                            nc.sync.dma_start(wsl[:], wq[4 * h + m])
                            pq = p2ps.tile([128, T_OWN], F32, tag="pq")
                            mm_chunks(pq, wsl, xk[:, :, T_KVP - T_OWN:], T_OWN)
                            nc.vector.tensor_copy(qfb[:, m], pq[:])
                        qb = p2t.tile([128, 4, T_OWN], BF16, tag="qb")
                        tq = p2t.tile([128, T_OWN], BF16, tag="tq")
                        for qh in range(2):
                            rope((qb[:, 2 * qh], qb[:, 2 * qh + 1]),
                                 qfb[:, 2 * qh], qfb[:, 2 * qh + 1],
                                 csq[:], snq[:], tq[:])
                        nc.sync.dma_start(qscr[h], qb[:])

            # =========== phase 3: attention ============================
            cm_p46 = _run(name="p46", bufs=1)
            p46 = cm_p46.__enter__()
            r2n = p46.tile([128, KC, T_OWN], BF16, tag="r2n")
            macc = p46.tile([128, KC, T_OWN], F32, tag="macc")
            inv_a = p46.tile([128, T_OWN], F32, tag="inv_a")
            cm_p3acc = _run(name="p3acc", bufs=1)
            p3acc = cm_p3acc.__enter__()
            with _run(name="p3c", bufs=1) as p3c, \
                 _run(name="p3h", bufs=2) as p3h, \
                 _run(name="p3t", bufs=3) as p3t, \
                 _run(name="p3ps", bufs=2, space="PSUM") as p3ps, \
                 _run(name="p3pa", bufs=1, space="PSUM") as p3pa:
                attn = p3acc.tile([128, HC, T_OWN], BF16)
                msk = p3c.tile([128, KT, T_OWN], F32)
                nc.sync.dma_start(msk[:], mask[:])

                for h in range(KV):
                    kb = p3h.tile([128, 2, T_KVP], BF16, tag="kb")
                    nc.sync.dma_start(kb[:], kscr[h])
                    vb = p3h.tile([128, 2, T_KVP], BF16, tag="vb")
                    nc.sync.dma_start(vb[:], vscr[h])
                    qb = p3h.tile([128, 4, T_OWN], BF16, tag="qb")
                    nc.sync.dma_start(qb[:], qscr[h])

                    # v -> token-major via PE transpose
                    vt = p3h.tile([128, KT, 2, 128], BF16, tag="vt")
                    for t in range(KT):
                        pt = p3ps.tile([128, 2, 128], BF16, tag="pt")
                        for d in range(2):
                            nc.tensor.transpose(
                                pt[:, d], vb[:, d, ts(t, 128)], ident[:])
                        nc.vector.tensor_copy(vt[:, t], pt[:])

                    for qh in range(2):
                        pa3 = p3pa.tile([128, 3, T_OWN], F32, tag="pa3")
                        pa = pa3[:, 0:2]
                        pss = pa3[:, 2]
                        for t in range(KT):
                            ps_s = p3ps.tile([128, T_OWN], F32, tag="ps")
                            for d in range(2):
                                nc.tensor.matmul(
                                    ps_s[:], kb[:, d, ts(t, 128)],
                                    qb[:, 2 * qh + d],
                                    start=(d == 0), stop=(d == 1))
                            sc = p3t.tile([128, T_OWN], F32, tag="sc")
                            nc.scalar.activation(sc[:], ps_s[:], AF.Tanh,
                                                 scale=1.0 / LOGIT_CAP)
                            nc.vector.scalar_tensor_tensor(
                                sc[:], sc[:], LOGIT_CAP, msk[:, t],
                                ALU.mult, ALU.add)
                            ex = p3t.tile([128, T_OWN], BF16, tag="ex")
                            nc.scalar.activation(ex[:], sc[:], AF.Exp)
                            for d in range(2):
                                nc.tensor.matmul(
                                    pa[:, d], vt[:, t, d], ex[:],
                                    start=(t == 0), stop=(t == KT - 1))
                            nc.tensor.matmul(pss[:], ones[:], ex[:],
                                             start=(t == 0),
                                             stop=(t == KT - 1))
                        inv = p3t.tile([128, T_OWN], F32, tag="inv")
                        nc.vector.reciprocal(inv[:], pss[:])
                        hh = 2 * h + qh
                        for d in range(2):
                            nc.vector.tensor_tensor(
                                attn[:, 2 * hh + d], pa[:, d], inv[:],
                                ALU.mult)

            # ===== phases 4-6 =============================================
            NQ = 4
            ICQ = IC // NQ
            at_scr = dscr.tile([KC, 128, T_OWN], F32, tag="at_scr")
            if True:
                # --- 4a: o_proj -> at (DRAM) + post-attn stats ------------
                with _run(name="p4a", bufs=1) as p4a, \
                     _run(name="p4w", bufs=3) as p4w, \
                     _run(name="p4t", bufs=3) as p4t, \
                     _run(name="p4ps", bufs=3, space="PSUM") as p4ps, \
                     _run(name="p4pss", bufs=1, space="PSUM") as p4pss:
                    iwpa = p4a.tile([128, KC, 128], BF16, tag="iwpa")
                    nc.sync.dma_start(iwpa[:], iw_pa[:])
                    pssq = p4pss.tile([128, T_OWN], F32)
                    for m in range(KC):
                        wsl = p4w.tile([128, HC, 128], BF16, tag="wo")
                        nc.sync.dma_start(wsl[:], wo[m])
                        pp = p4ps.tile([128, T_OWN], F32, tag="po")
                        for c in range(HC):
                            nc.tensor.matmul(pp[:], wsl[:, c], attn[:, c],
                                             start=(c == 0),
                                             stop=(c == HC - 1))
                        atc = p4t.tile([128, T_OWN], F32, tag="atc")
                        nc.scalar.copy(atc[:], pp[:])
                        nc.sync.dma_start(at_scr[m], atc[:])
                        sq = p4t.tile([128, T_OWN], BF16, tag="sq4")
                        nc.scalar.activation(sq[:], pp[:], AF.Square)
                        nc.tensor.matmul(pssq[:], iwpa[:, m], sq[:],
                                         start=(m == 0), stop=(m == KC - 1))
                    rms = p4t.tile([128, T_OWN], F32, tag="rms4")
                    nc.scalar.activation(rms[:], pssq[:], AF.Sqrt,
                                         scale=1.0 / HID, bias=epsb[:])
                    nc.vector.reciprocal(inv_a[:], rms[:])
                cm_p3acc.__exit__(None, None, None)  # free attn

                # --- 4b: residual, r_out, pre-ff norm, r2n ----------------
                with _run(name="p4b", bufs=1) as p4b, \
                     _run(name="p4bt", bufs=3) as p4bt, \
                     _run(name="p4bps", bufs=1, space="PSUM") as p4bps:
                    xo = p4b.tile([128, KC, T_OWN], F32, tag="xo")
                    nc.sync.dma_start(xo[:], xown[:])
                    ps2 = p4bps.tile([128, T_OWN], F32)
                    for m in range(KC):
                        atc = p4bt.tile([128, T_OWN], F32, tag="atc2")
                        nc.sync.dma_start(atc[:], at_scr[m])
                        nc.vector.tensor_tensor(atc[:], atc[:], inv_a[:],
                                                ALU.mult)
                        nc.vector.tensor_add(xo[:, m], xo[:, m], atc[:])
                        sq = p4bt.tile([128, T_OWN], BF16, tag="sq5")
                        nc.scalar.activation(sq[:], xo[:, m], AF.Square)
                        nc.tensor.matmul(ps2[:], ones[:], sq[:],
                                         start=(m == 0), stop=(m == KC - 1))
                    nc.sync.dma_start(r_out[:], xo[:])
                    rms2 = p4bt.tile([128, T_OWN], F32, tag="rms5")
                    nc.scalar.activation(rms2[:], ps2[:], AF.Sqrt,
                                         scale=1.0 / HID, bias=epsb[:])
                    inv2 = p4bt.tile([128, T_OWN], F32, tag="inv5")
                    nc.vector.reciprocal(inv2[:], rms2[:])
                    nc.vector.tensor_tensor(
                        r2n[:], xo[:],
                        inv2[:, None, :].to_broadcast([128, KC, T_OWN]),
                        ALU.mult)

                # --- 5+6: MLP in 4 inter-quarters -------------------------
                with _run(name="p5q", bufs=1) as p5q, \
                     _run(name="p5w", bufs=3) as p5w, \
                     _run(name="p6w", bufs=2) as p6w, \
                     _run(name="p5t", bufs=3) as p5t, \
                     _run(name="p5ps", bufs=2, space="PSUM") as p5ps, \
                     _run(name="p6ps", bufs=3, space="PSUM") as p6ps:
                    for q in range(NQ):
                        itr = p5q.tile([128, ICQ, T_OWN], BF16, tag="itr")
                        for mi in range(ICQ):
                            m = q * ICQ + mi
                            wg_s = p5w.tile([128, KC, 128], BF16, tag="wg")
                            nc.sync.dma_start(wg_s[:], wg[m])
                            wu_s = p5w.tile([128, KC, 128], BF16, tag="wu")
                            nc.sync.dma_start(wu_s[:], wu[m])
                            pg = p5ps.tile([128, T_OWN], F32, tag="pg")
                            pu = p5ps.tile([128, T_OWN], F32, tag="pu")
                            for c in range(KC):
                                nc.tensor.matmul(pg[:], wg_s[:, c], r2n[:, c],
                                                 start=(c == 0),
                                                 stop=(c == KC - 1))
                            for c in range(KC):
                                nc.tensor.matmul(pu[:], wu_s[:, c], r2n[:, c],
                                                 start=(c == 0),
                                                 stop=(c == KC - 1))
                            gl = p5t.tile([128, T_OWN], BF16, tag="gl")
                            nc.scalar.activation(gl[:], pg[:],
                                                 AF.Gelu_apprx_tanh)
                            ub = p5t.tile([128, T_OWN], BF16, tag="ub")
                            nc.vector.tensor_copy(ub[:], pu[:])
                            nc.vector.tensor_tensor(itr[:, mi], gl[:], ub[:],
                                                    ALU.mult)
                        for m in range(KC):
                            wsl = p6w.tile([128, ICQ, 128], BF16, tag="wd")
                            nc.sync.dma_start(wsl[:], wd[m, :, ts(q, ICQ), :])
                            pp = p6ps.tile([128, T_OWN], F32, tag="pd")
                            for c in range(ICQ):
                                nc.tensor.matmul(pp[:], wsl[:, c], itr[:, c],
                                                 start=(c == 0),
                                                 stop=(c == ICQ - 1))
                            if q == 0:
                                nc.scalar.copy(macc[:, m], pp[:])
                            else:
                                nc.vector.tensor_add(macc[:, m], macc[:, m],
                                                     pp[:])

                    # post-ff norm over macc
                    with _run(name="p6t", bufs=3) as p6t, \
                         _run(name="p6big", bufs=1) as p6big, \
                         _run(name="p6pss", bufs=1, space="PSUM") as p6pss:
                        iwpf = p6big.tile([128, KC, 128], BF16, tag="iwpf")
                        nc.sync.dma_start(iwpf[:], iw_pf[:])
                        ps3 = p6pss.tile([128, T_OWN], F32)
                        for m in range(KC):
                            sq = p6t.tile([128, T_OWN], BF16, tag="sq6")
                            nc.scalar.activation(sq[:], macc[:, m], AF.Square)
                            nc.tensor.matmul(ps3[:], iwpf[:, m], sq[:],
                                             start=(m == 0), stop=(m == KC - 1))
                        rms3 = p6t.tile([128, T_OWN], F32, tag="rms6")
                        nc.scalar.activation(rms3[:], ps3[:], AF.Sqrt,
                                             scale=1.0 / HID, bias=epsb[:])
                        inv3 = p6t.tile([128, T_OWN], F32, tag="inv6")
                        nc.vector.reciprocal(inv3[:], rms3[:])
                        for m in range(KC):
                            nc.vector.tensor_tensor(macc[:, m], macc[:, m],
                                                    inv3[:], ALU.mult)
                            nc.sync.dma_start(x_out[:, m], macc[:, m])
                cm_p46.__exit__(None, None, None)

    nc.compile()
    return nc


_PROGRAM = None


def _get_program():
    global _PROGRAM
    if _PROGRAM is None:
        _PROGRAM = _build_program()
    return _PROGRAM


# ---------------------------------------------------------------------------
# Host-side input prep
# ---------------------------------------------------------------------------
def _fm(x):
    """[T, D] -> feature-major [128, D//128, T]"""
    D = x.shape[1]
    return np.ascontiguousarray(
        x.T.reshape(D // 128, 128, -1).transpose(1, 0, 2))


def _wtiles(w):
    """[K, M] -> [M//128, 128, K//128, 128] lhsT-tile-major"""
    K, M = w.shape
    return np.ascontiguousarray(
        w.reshape(K // 128, 128, M // 128, 128).transpose(2, 1, 0, 3))


def kernel(hidden_states, positions, in_ln_w, post_attn_ln_w, pre_ff_ln_w,
           post_ff_ln_w, q_w, k_w, v_w, o_w, gate_w, up_w, down_w):
    global LAST_EXEC_NS
    nc = _get_program()

    f32 = np.float32
    hs = np.asarray(hidden_states, f32)
    pos = np.asarray(positions, np.int32)

    # ---- shared (weight) inputs, computed once ----
    s_in = (1.0 + np.asarray(in_ln_w, f32))[:, None]
    wq_h = _wtiles((np.asarray(q_w, f32) * s_in * SCALING).astype(nbf16))
    wk_h = _wtiles((np.asarray(k_w, f32) * s_in).astype(nbf16))
    wv_h = _wtiles((np.asarray(v_w, f32) * s_in).astype(nbf16))
    wo_h = _wtiles((np.asarray(o_w, f32) *
                    (1.0 + np.asarray(post_attn_ln_w, f32))[None, :]
                    ).astype(nbf16))
    s_ff = (1.0 + np.asarray(pre_ff_ln_w, f32))[:, None]
    wg_h = _wtiles((np.asarray(gate_w, f32) * s_ff).astype(nbf16))
    wu_h = _wtiles((np.asarray(up_w, f32) * s_ff).astype(nbf16))
    wd_h = _wtiles((np.asarray(down_w, f32) *
                    (1.0 + np.asarray(post_ff_ln_w, f32))[None, :]
                    ).astype(nbf16))

    def iw2(w):
        v = (1.0 / (1.0 + np.asarray(w, f32)) ** 2).reshape(KC, 128)
        return np.ascontiguousarray(
            np.broadcast_to(v.T[:, :, None], (128, KC, 128))).astype(nbf16)

    iwpa_h = iw2(post_attn_ln_w)
    iwpf_h = iw2(post_ff_ln_w)

    inv_freq = 1.0 / (ROPE_BASE ** (np.arange(0, DH, 2, dtype=f32) / DH))

    in_maps = []
    metas = []
    for c in range(N_CORES):
        b, s = divmod(c, N_CORES // B)
        o0, o1 = T_OWN * s, T_OWN * (s + 1)
        kv0 = o1 - T_KVP  # may be negative -> zero pad
        xo = hs[b, o0:o1]                               # [512, HID]
        xv = np.zeros((T_KVP, HID), f32)
        src0 = max(kv0, 0)
        xv[src0 - kv0:] = hs[b, src0:o1]
        pq = pos[b, o0:o1].astype(f32)                  # [512]
        pk = np.zeros(T_KVP, f32)
        pk[src0 - kv0:] = pos[b, src0:o1].astype(f32)

        ang_q = pq[:, None] * inv_freq[None, :]         # [512, 128]
        ang_k = pk[:, None] * inv_freq[None, :]

        # mask[k, q]: valid iff kpos>=0 & kpos<=qpos & qpos-kpos<WINDOW
        kpos = np.arange(kv0, o1)
        qpos = np.arange(o0, o1)
        d = qpos[None, :] - kpos[:, None]
        valid = (kpos[:, None] >= 0) & (d >= 0) & (d < WINDOW)
        m = np.where(valid, -LOGIT_CAP, NEG - LOGIT_CAP).astype(f32)
        m3 = np.ascontiguousarray(
            m.reshape(KT, 128, T_OWN).transpose(1, 0, 2))

        in_maps.append({
            "xkv": _fm(xv).astype(nbf16),
            "xown": _fm(xo),
            "wq": wq_h, "wk": wk_h, "wv": wv_h, "wo": wo_h,
            "wg": wg_h, "wu": wu_h, "wd": wd_h,
            "iw_pa": iwpa_h, "iw_pf": iwpf_h,
            "cos_q": np.ascontiguousarray(np.cos(ang_q).T).astype(nbf16),
            "sin_q": np.ascontiguousarray(np.sin(ang_q).T).astype(nbf16),
            "cos_k": np.ascontiguousarray(np.cos(ang_k).T).astype(nbf16),
            "sin_k": np.ascontiguousarray(np.sin(ang_k).T).astype(nbf16),
            "cos_ko": np.ascontiguousarray(np.cos(ang_q).T).astype(f32),
            "sin_ko": np.ascontiguousarray(np.sin(ang_q).T).astype(f32),
            "mask": m3,
        })
        metas.append((b, o0, o1))

    res = run_bass_kernel_spmd(nc, in_maps, core_ids=list(range(N_CORES)),
                               trace=TRACE,
                               trace_cores=[0] if TRACE else None)
    LAST_EXEC_NS = res.exec_time_ns

    # ---- assemble full outputs ----
    x_full = np.zeros((B, S, HID), f32)
    r_full = np.zeros((B, S, HID), f32)
    kv_full = np.zeros((2, B, S, KV, DH), f32)

    def unfm(t):
        # [128, C, T] -> [T, 128*C]
        return t.transpose(1, 0, 2).reshape(-1, t.shape[2]).T

    for c in range(N_CORES):
        b, o0, o1 = metas[c]
        r = res.results[c]
        x_full[b, o0:o1] = unfm(r["x_out"])
        r_full[b, o0:o1] = unfm(r["r_out"])
        kv_full[0, b, o0:o1] = unfm(r["k_out"]).reshape(T_OWN, KV, DH)
        kv_full[1, b, o0:o1] = unfm(r["v_out"]).reshape(T_OWN, KV, DH)

    return x_full, r_full, kv_full
